# revision 8
# baseline (speedup 1.0000x reference)
"""GAT (3-layer, PyG-style) forward on 8 Trainium2 NeuronCores.

v4 strategy (quarter-tables):
  - Nodes assigned to cores by id%8; each core's rank space [0,PC) split in 4
    quarters of Q ranks. Sources are greedily "colored" into quarters to
    balance each dst's in-edges across quarters; dsts are z-order clustered
    within (core, quarter) by their 4-vector quarter-degree profile.
  - Per layer, per quarter q, a Shared table tabq[l][q] = AllGather over cores
    of that quarter's staged rows ([h 64|AD 4|E1 4|E2 4] bf16, 256B rows).
    Chunk AGs fire progressively inside the previous layer's edge loop, so
    only the last chunk is exposed. Gather windows = one whole quarter-table
    (8*QB rows <= 32767, int16-addressable).
  - Tiles are paired K-balanced within each quarter; per tile the slot count
    K is uniform across the 4 quarters, so a unit's gathered slots form 4
    equal s-blocks -> all hot DVE ops are flat unit-stride bf16 (2x mode):
    one t12 mult (vs ACT-expanded D12), one max, block-folds for value and
    denominator sums. The per-slot e is ACT-expanded to 64 wide, and the
    value multiply runs in-place at 2x.
  - exp(leaky_relu(as+ad)) == max(exp(as)exp(ad), exp(.2as)exp(.2ad)).
  - Self contributions precomputed densely per layer; per-unit post ops,
    next-layer node matmul fused; final pooling matmul + AllReduce + MLP.
"""

import sys

sys.path.insert(0, "/opt/trn_rl_repo")

import numpy as np
import ml_dtypes

BF16 = ml_dtypes.bfloat16

MAXJ_CALL = 28  # <=3584 idx per dma_gather (ring w/ 64KB scratch)
NQ = 4          # SWDGE queues
SW = 76         # staged row cols [h 64|AD 4|E1 4|E2 4]


# ----------------------------------------------------------------- host prep

def _color_sources(src, dst, NSTAR, rng):
    """Greedy 4-coloring of sources balancing per-dst quarter counts."""
    E = len(src)
    order_e = np.argsort(src, kind='stable')
    src_s, dst_s = src[order_e], dst[order_e]
    uniq, starts = np.unique(src_s, return_index=True)
    ends = np.append(starts[1:], E)
    outdeg_u = ends - starts
    proc = np.argsort(-outdeg_u, kind='stable')
    d_tot = np.bincount(dst, minlength=NSTAR)
    ideal = (d_tot + 3) // 4

    cnt = np.zeros((NSTAR, 4), np.int32)
    color = (np.arange(NSTAR) % 4).astype(np.int8)
    for i in proc:
        u = uniq[i]
        ds = dst_s[starts[i]:ends[i]]
        c_over = np.maximum(cnt[ds] + 1 - ideal[ds][:, None], 0).sum(0)
        c = int(np.argmin(c_over * 1000 + cnt[ds].sum(0)))
        color[u] = c
        cnt[ds, c] += 1
    for _ in range(2):
        for i in proc:
            u = uniq[i]
            ds = dst_s[starts[i]:ends[i]]
            cnt[ds, color[u]] -= 1
            c_over = np.maximum(cnt[ds] + 1 - ideal[ds][:, None], 0).sum(0)
            c = int(np.argmin(c_over * 1000 + cnt[ds].sum(0)))
            color[u] = c
            cnt[ds, c] += 1
    return color


def _zorder(prof):
    p = np.minimum(prof, 31).astype(np.int64)
    z = np.zeros(len(p), np.int64)
    for bit in range(5):
        for d in range(4):
            z |= ((p[:, d] >> bit) & 1) << (bit * 4 + d)
    return np.argsort(z, kind='stable')


def _prep(x, edge_index, batch, n_graphs):
    N = x.shape[0]
    NC = 8
    PC = int(np.ceil(N / NC / 512)) * 512
    NSTAR = NC * PC
    TILES = PC // 128
    Q = PC // 4
    QB = Q + 128
    TPQ = TILES // 4
    DUMMY_LOC = Q                     # row Q of core-0 block in each tabq

    src = edge_index[0].astype(np.int64)
    dst = edge_index[1].astype(np.int64)
    rng = np.random.default_rng(0)

    core_of = np.full(NSTAR, -1, np.int64)
    core_of[:N] = np.arange(N) % NC

    color = _color_sources(src, dst, NSTAR, rng)

    # per (core, color) capacity Q: rebalance moving low-outdeg nodes
    odeg_all = np.bincount(src, minlength=NSTAR)
    for c in range(NC):
        ids = np.where(core_of == c)[0]
        cols = color[ids].astype(np.int64)
        for _ in range(12):
            cc = np.bincount(cols, minlength=4)
            if (cc <= Q).all():
                break
            over = int(np.argmax(cc))
            under = int(np.argmin(cc))
            n_move = min(cc[over] - Q, Q - cc[under])
            cand = np.where(cols == over)[0]
            take = cand[np.argsort(odeg_all[ids[cand]])[:n_move]]
            cols[take] = under
        color[ids] = cols

    degq = np.zeros((NSTAR, 4), np.int64)
    np.add.at(degq, (dst, color[src]), 1)

    # rank within (core, quarter): z-order of profile; pads fill the rest
    rank_of = np.full(NSTAR, -1, np.int64)
    for c in range(NC):
        ids = np.where(core_of == c)[0]
        cols = color[ids]
        for q in range(4):
            sub = ids[cols == q]
            o = _zorder(degq[sub])
            rank_of[sub[o]] = q * Q + np.arange(len(sub))

    # per-core-tile per-s max, then cross-core & cross-s max -> K_round[t]
    tile_K = np.zeros((NC, TILES, 4), np.int64)
    for c in range(NC):
        ids = np.where(core_of == c)[0]
        t = rank_of[ids] // 128
        prof = degq[ids]
        for s in range(4):
            np.maximum.at(tile_K[c, :, s], t, prof[:, s])
    K_round = tile_K.max(axis=(0, 2))          # [TILES], s-uniform

    # units: K-balanced pairing within each quarter
    units = []                                 # (tA, tB|None, KA, KB)
    unit_quarter = []
    for q in range(4):
        tq = np.arange(q * TPQ, (q + 1) * TPQ)
        order = tq[np.argsort(K_round[tq], kind='stable')]
        for i in range(TPQ // 2):
            tA, tB = int(order[i]), int(order[TPQ - 1 - i])
            units.append((tA, tB, int(K_round[tA]), int(K_round[tB])))
            unit_quarter.append(q)
        if TPQ % 2:
            tA = int(order[TPQ // 2])
            units.append((tA, None, int(K_round[tA]), 0))
            unit_quarter.append(q)
    NU = len(units)
    # AG trigger unit index per quarter (last unit of each quarter)
    ag_after = {}
    for ui, q in enumerate(unit_quarter):
        ag_after[q] = ui

    # ---- slot lists ------------------------------------------------------
    slots = [[np.full((128, 4 * int(K_round[t])), DUMMY_LOC, np.int16)
              for t in range(TILES)] for c in range(NC)]
    # layout per tile: s-major [s0 K | s1 K | s2 K | s3 K]
    dcore = core_of[dst]
    drank = rank_of[dst]
    scol = color[src].astype(np.int64)
    loc = (core_of[src] * QB + (rank_of[src] - scol * Q)).astype(np.int64)
    assert loc.min() >= 0 and loc.max() < NC * QB <= 32767
    order = np.lexsort((scol, drank, dcore))
    kfill = np.zeros((NSTAR, 4), np.int64)
    dco, dro, sco, lco, dsto = (dcore[order], drank[order], scol[order],
                                loc[order], dst[order])
    Kt = K_round.astype(np.int64)
    for i in range(len(dco)):
        c = dco[i]
        r = dro[i]
        t = r // 128
        p = r % 128
        s = sco[i]
        k = kfill[dsto[i], s]
        slots[c][t][p, s * Kt[t] + k] = lco[i]
        kfill[dsto[i], s] = k + 1

    # ---- calls (uniform across cores) -----------------------------------
    calls = []           # (ui, s, joff_unit, cj)
    unit_J = []
    for ui, (tA, tB, KA, KB) in enumerate(units):
        KU = KA + KB
        for s in range(4):
            o = 0
            while o < KU:
                cj = min(MAXJ_CALL, KU - o)
                calls.append((ui, s, s * KU + o, cj))
                o += cj
        unit_J.append(4 * KU)

    def wrap16(ix):
        a = ix.reshape(-1, 16).T
        return np.tile(a, (8, 1))

    tile_cols = []       # (idx col offset, ncols) per unit
    off = 0
    for ui in range(NU):
        ncols = 128 * unit_J[ui] // 16
        tile_cols.append((off, ncols))
        off += ncols

    idx_cores = []
    for c in range(NC):
        parts = []
        for (ui, s, joff, cj) in calls:
            tA, tB, KA, KB = units[ui]
            KU = KA + KB
            blkA = slots[c][tA][:, s * KA:(s + 1) * KA]
            if tB is not None:
                blkB = slots[c][tB][:, s * KB:(s + 1) * KB]
                blk = np.concatenate([blkA, blkB], axis=1)
            else:
                blk = blkA
            js = joff - s * KU
            sub = blk[:, js:js + cj]
            ix = sub.T.reshape(-1).astype(np.int16)
            parts.append(wrap16(ix).astype(np.int16))
        idx_cores.append(np.concatenate(parts, axis=1)
                         if parts else np.zeros((128, 0), np.int16))
    idx_all = np.stack(idx_cores)

    pi_of = np.full(NSTAR, -1, np.int64)
    pi_of[:N] = core_of[:N] * PC + rank_of[:N]
    inv_pi = np.full(NSTAR, -1, np.int64)
    inv_pi[pi_of[:N]] = np.arange(N)

    batch = np.asarray(batch, np.int64)
    goh = np.zeros((NC, PC, n_graphs), np.float32)
    for c in range(NC):
        b = np.full(PC, -1, np.int64)
        iv = inv_pi[c * PC:(c + 1) * PC]
        m = iv >= 0
        b[m] = batch[iv[m]]
        valid = b >= 0
        goh[c, np.arange(PC)[valid], b[valid]] = 1.0
    counts = np.maximum(np.bincount(batch, minlength=n_graphs), 1.0)

    return dict(
        N=N, NC=NC, PC=PC, NSTAR=NSTAR, TILES=TILES, Q=Q, QB=QB, TPQ=TPQ,
        K_round=K_round, units=units, unit_J=unit_J, calls=calls,
        tile_cols=tile_cols, idx_all=idx_all, inv_pi=inv_pi, pi_of=pi_of,
        goh=goh, counts=counts, ag_after=ag_after,
    )


def _augment_w(W, a_s, a_d, heads=4, hid=16):
    """[F, H*C] weights -> [F, 72] augmented (bf16): [W | Wad | Was]."""
    F = W.shape[0]
    Wr = W.reshape(F, heads, hid)
    was = np.einsum("fhc,hc->fh", Wr, a_s)
    wad = np.einsum("fhc,hc->fh", Wr, a_d)
    out = np.concatenate([W, wad, was], axis=1).astype(np.float32)
    return out.astype(BF16)


# ------------------------------------------------------------- kernel build

def _build(meta, n_graphs, f_in, run_layers=3):
    import concourse.tile as tile
    from concourse import bacc, mybir
    from concourse.masks import make_identity

    NC, PC, TILES = meta["NC"], meta["PC"], meta["TILES"]
    Q, QB, TPQ = meta["Q"], meta["QB"], meta["TPQ"]
    TROWS = NC * QB
    units, unit_J, calls = meta["units"], meta["unit_J"], meta["calls"]
    tile_cols, ag_after = meta["tile_cols"], meta["ag_after"]
    NU = len(units)
    TOTC = meta["idx_all"].shape[2]
    G = n_graphs
    f32, bf16, i16 = mybir.dt.float32, mybir.dt.bfloat16, mybir.dt.int16
    AX, ALU = mybir.AxisListType, mybir.AluOpType
    ACT = mybir.ActivationFunctionType

    nc = bacc.Bacc(None, target_bir_lowering=False, debug=False,
                   num_devices=NC, num_swdge_queues=NQ,
                   dynamic_dma_scratch_size=65536)

    # ---- I/O ----
    xT = nc.dram_tensor("xT", [f_in, PC], bf16, kind="ExternalInput")
    idx_in = nc.dram_tensor("idx", [128, max(TOTC, 8)], i16,
                            kind="ExternalInput")
    goh_in = nc.dram_tensor("goh", [PC, G], bf16, kind="ExternalInput")
    w1 = nc.dram_tensor("w1", [f_in, 72], bf16, kind="ExternalInput")
    w2 = nc.dram_tensor("w2", [64, 72], bf16, kind="ExternalInput")
    w3 = nc.dram_tensor("w3", [16, 72], bf16, kind="ExternalInput")
    b1r = nc.dram_tensor("b1r", [128, 128], f32, kind="ExternalInput")
    b2r = nc.dram_tensor("b2r", [128, 32], f32, kind="ExternalInput")
    b3r = nc.dram_tensor("b3r", [128, 32], f32, kind="ExternalInput")
    cntr = nc.dram_tensor("cntr", [16, G], f32, kind="ExternalInput")
    statsT = nc.dram_tensor("statsT", [16, G], f32, kind="ExternalInput")
    fw1 = nc.dram_tensor("fw1", [32, 32], f32, kind="ExternalInput")
    fb1 = nc.dram_tensor("fb1", [32, 1], f32, kind="ExternalInput")
    fw2 = nc.dram_tensor("fw2", [32, 16], f32, kind="ExternalInput")
    fb2 = nc.dram_tensor("fb2", [16, 1], f32, kind="ExternalInput")
    fw3 = nc.dram_tensor("fw3", [16, 1], f32, kind="ExternalInput")
    fb3 = nc.dram_tensor("fb3", [1, 1], f32, kind="ExternalInput")
    dumr = nc.dram_tensor("dumr", [1, 128], bf16, kind="ExternalInput")
    out_t = nc.dram_tensor("out", [1, G], f32, kind="ExternalOutput")

    LIN = [f_in, 64, 16]
    LOUT = [64, 16, 16]

    with tile.TileContext(nc, num_cores=NC) as tc:
        with (
            tc.tile_pool(name="dram", bufs=1, space="DRAM") as dpool,
            tc.tile_pool(name="consts", bufs=1) as cpool,
            tc.tile_pool(name="nodein", bufs=2) as npool,
            tc.tile_pool(name="psum", bufs=2, space="PSUM") as ppool,
            tc.tile_pool(name="pst", bufs=2, space="PSUM") as ptpool,
            tc.tile_pool(name="mlpp", bufs=1, space="PSUM") as mpool,
            tc.tile_pool(name="stall", bufs=1) as stpool,
            tc.tile_pool(name="selfp", bufs=1) as sepool,
            tc.tile_pool(name="gat", bufs=2) as gpool,
            tc.tile_pool(name="e64p", bufs=2) as vpool,
            tc.tile_pool(name="idxp", bufs=4) as ipool,
            tc.tile_pool(name="edge", bufs=3) as epool,
            tc.tile_pool(name="escr", bufs=2) as e1pool,
            tc.tile_pool(name="poolacc", bufs=1, space="PSUM") as papool,
            tc.tile_pool(name="head", bufs=1) as hpool,
        ):
            tabmine = [dpool.tile([4 * QB, 128], bf16, tag=f"tm{l}",
                                  name=f"tabmine{l}") for l in range(3)]
            tabq = [[dpool.tile([TROWS, 128], bf16, tag=f"tab{l}_{s}",
                                name=f"table{l}_{s}", addr_space="Shared")
                     for s in range(4)] for l in range(3)]
            cc_in = dpool.tile([16, G], f32, tag="ccin")
            cc_out = dpool.tile([16, G], f32, tag="ccout",
                                addr_space="Shared")

            ident = cpool.tile([128, 128], bf16)
            make_identity(nc, ident[:])
            wsb = []
            for l, wt in enumerate((w1, w2, w3)):
                t = cpool.tile([LIN[l], 72], bf16, tag=f"w{l}", name=f"wsb{l}")
                nc.sync.dma_start(t[:], wt[:, :])
                wsb.append(t)
            brep = []
            for l, (bt, w2_) in enumerate(((b1r, 128), (b2r, 32), (b3r, 32))):
                t = cpool.tile([128, w2_], f32, tag=f"b{l}", name=f"bsb{l}")
                nc.sync.dma_start(t[:], bt[:, :])
                brep.append(t)
            dum_sb = cpool.tile([1, 128], bf16)
            nc.sync.dma_start(dum_sb[:], dumr[:, :])

            st_bufs = [stpool.tile([128, TILES * SW], bf16, tag=f"sta{l}",
                                   name=f"stall{l}") for l in range(2)]
            st_all = [st_bufs[0], st_bufs[1], st_bufs[0]]

            pool_ps = papool.tile([16, G], f32)
            goh_sb = stpool.tile([128, TILES * G], bf16, tag="gohsb")
            gview = goh_in[:].rearrange("(t p) g -> p t g", p=128)
            nc.sync.dma_start(
                goh_sb[:].rearrange("p (t g) -> p t g", g=G), gview)

            qctr = [0]

            def gather_queue():
                q = qctr[0] % NQ
                qctr[0] += 1
                return q

            def elu_out(x_ap, out_ap, w):
                # elu(x) = relu(x) + exp(-relu(-x)) - 1
                t1 = e1pool.tile([128, w], bf16, tag="el1")
                t2 = e1pool.tile([128, w], bf16, tag="el2")
                nc.scalar.activation(t1[:], x_ap, ACT.Relu, scale=-1.0)
                nc.scalar.activation(t1[:], t1[:], ACT.Exp, scale=-1.0)
                nc.scalar.activation(t2[:], x_ap, ACT.Relu)
                nc.vector.tensor_scalar_add(t1[:], t1[:], -1.0)
                nc.vector.tensor_tensor(out=out_ap, in0=t1[:], in1=t2[:],
                                        op=ALU.add)

            def node_tile(l, t, lhsT_ap):
                """row_l[tile t] = [h|AD|E1|E2] -> st_all[l] + tabmine[l]."""
                ps = ppool.tile([128, 72], f32, tag="nps")
                nc.tensor.matmul(ps[:], lhsT_ap, wsb[l][:],
                                 start=True, stop=True)
                sl = st_all[l][:, t * SW:(t + 1) * SW]
                nc.scalar.copy(sl[:, 0:68], ps[:, 0:68])
                nc.scalar.activation(sl[:, 68:72], ps[:, 68:72], ACT.Exp)
                nc.scalar.activation(sl[:, 72:76], ps[:, 68:72], ACT.Exp,
                                     scale=0.2)
                q = t // TPQ
                row0 = q * QB + (t % TPQ) * 128
                nc.sync.dma_start(tabmine[l][row0:row0 + 128, 0:SW], sl)

            def ag_chunk(l, q):
                nc.sync.dma_start(tabmine[l][q * QB + Q:q * QB + Q + 1, :],
                                  dum_sb[:])
                nc.gpsimd.collective_compute(
                    "AllGather", mybir.AluOpType.bypass,
                    replica_groups=[list(range(NC))],
                    ins=[tabmine[l][q * QB:(q + 1) * QB, :]],
                    outs=[tabq[l][q].opt()])

            # ---------------- layer 0 node phase --------------------------
            for q in range(4):
                t0, t1_ = q * TPQ, (q + 1) * TPQ
                xin = npool.tile([f_in, TPQ * 128], bf16, tag="xin")
                nc.sync.dma_start(xin[:], xT[:, t0 * 128:t1_ * 128])
                for t in range(t0, t1_):
                    node_tile(0, t, xin[:, (t - t0) * 128:(t - t0 + 1) * 128])
                ag_chunk(0, q)

            for l in range(run_layers):
                W = LOUT[l]
                sa = st_all[l]
                sa3 = sa[:].rearrange("p (r c) -> p r c", c=SW)

                # ---- bulk self/dst precompute --------------------------
                d12 = sepool.tile([128, TILES * 8], bf16, tag="d12")
                d123 = d12[:].rearrange("p (r v) -> p r v", v=8)
                nc.scalar.activation(d123[:, :, 0:4], sa3[:, :, 64:68],
                                     ACT.Exp)
                nc.scalar.activation(d123[:, :, 4:8], sa3[:, :, 64:68],
                                     ACT.Exp, scale=0.2)
                est = sepool.tile([128, TILES * 8], bf16, tag="est")
                est3 = est[:].rearrange("p (r v) -> p r v", v=8)
                nc.vector.tensor_tensor(out=est3[:, :, :],
                                        in0=sa3[:, :, 68:76],
                                        in1=d123[:, :, :], op=ALU.mult)
                es_all = sepool.tile([128, TILES * 4], f32, tag="esal")
                es3 = es_all[:].rearrange("p (r q) -> p r q", q=4)
                nc.vector.tensor_tensor(out=es3[:, :, :],
                                        in0=est3[:, :, 0:4],
                                        in1=est3[:, :, 4:8], op=ALU.max)
                sv_all = sepool.tile([128, TILES * 64], bf16, tag="sval")
                sv4 = sv_all[:].rearrange("p (r q c) -> p r q c", q=4, c=16)
                nc.vector.tensor_tensor(
                    out=sv4[:, :, :, :],
                    in0=sa3[:, :, 0:64].rearrange("p r (q c) -> p r q c",
                                                  c=16),
                    in1=es3.unsqueeze(3).to_broadcast([128, TILES, 4, 16]),
                    op=ALU.mult)

                # ---------------- edge phase ----------------------------
                def emit_post(ui, tiles_here, npr, U, dful):
                    recip = epool.tile([128, npr * 4], f32, tag="rec")
                    nc.vector.reciprocal(recip[:], dful[:])
                    if l > 0:
                        nc.vector.tensor_scalar_mul(recip[:], recip[:], 0.25)
                    o64 = epool.tile([128, npr * 64], f32, tag="o64")
                    nc.vector.tensor_tensor(
                        out=o64[:].rearrange("p (t q c) -> p t q c",
                                             q=4, c=16),
                        in0=U[:].rearrange("p (t q c) -> p t q c",
                                           q=4, c=16),
                        in1=recip[:].rearrange(
                            "p (t q) -> p t q", q=4).unsqueeze(
                            3).to_broadcast([128, npr, 4, 16]),
                        op=ALU.mult)
                    if l == 0:
                        nc.vector.tensor_tensor(
                            out=o64[:], in0=o64[:],
                            in1=brep[0][:, 0:npr * 64], op=ALU.add)
                        xnext = epool.tile([128, npr * 64], bf16, tag="xn")
                        elu_out(o64[:], xnext[:], npr * 64)
                        WX = 64
                    else:
                        o16 = epool.tile([128, npr * 16], f32, tag="o16")
                        nc.vector.tensor_reduce(
                            o16[:],
                            o64[:].rearrange("p (t q c) -> p t c q",
                                             q=4, c=16),
                            AX.X, ALU.add)
                        xnext = epool.tile([128, npr * 16], bf16, tag="xn16")
                        if l == 1:
                            nc.vector.tensor_tensor(
                                out=o16[:], in0=o16[:],
                                in1=brep[l][:, 0:npr * 16], op=ALU.add)
                            elu_out(o16[:], xnext[:], npr * 16)
                        else:
                            nc.vector.tensor_tensor(
                                out=xnext[:], in0=o16[:],
                                in1=brep[l][:, 0:npr * 16], op=ALU.add)
                        WX = 16

                    if l < 2:
                        for i, (t, K, ko) in enumerate(tiles_here):
                            pst = ptpool.tile([WX, 128], bf16, tag="pst")
                            nc.tensor.transpose(
                                out=pst[:],
                                in_=xnext[:, WX * i:WX * (i + 1)],
                                identity=ident[:])
                            stt = epool.tile([WX, 128], bf16, tag="stt")
                            nc.scalar.copy(stt[:], pst[:])
                            if run_layers > l + 1:
                                node_tile(l + 1, t, stt[:])
                    else:
                        for i, (t, K, ko) in enumerate(tiles_here):
                            nc.tensor.matmul(
                                pool_ps[:], xnext[:, 16 * i:16 * i + 16],
                                goh_sb[:, t * G:(t + 1) * G],
                                start=(ui == 0 and i == 0),
                                stop=(ui == NU - 1 and i == npr - 1))

                    if l < 2 and run_layers > l + 1:
                        for q in range(4):
                            if ag_after[q] == ui:
                                ag_chunk(l + 1, q)

                pend = None
                for ui in range(NU):
                    tA, tB, KA, KB = units[ui]
                    KU = KA + KB
                    J = 4 * KU
                    tiles_here = [(tA, KA, 0)] if tB is None else \
                        [(tA, KA, 0), (tB, KB, KA)]
                    npr = len(tiles_here)
                    coff, ncols = tile_cols[ui]

                    if J > 0:
                        it = ipool.tile([128, max(ncols, 8)], i16, tag="idx")
                        nc.sync.dma_start(it[:, 0:ncols],
                                          idx_in[:, coff:coff + ncols])
                        gat = gpool.tile([128, J * 128], bf16, tag="gat")
                        g3 = gat[:].rearrange("p (j e) -> p j e", e=128)
                        ccol = 0
                        for (ui2, s_, joff, cj) in calls:
                            if ui2 != ui:
                                continue
                            n_i = 128 * cj
                            nc.gpsimd.dma_gather(
                                g3[:, joff:joff + cj, :],
                                tabq[l][s_][0:TROWS, :],
                                it[:, ccol:ccol + n_i // 16],
                                n_i, n_i, 128,
                                queue_num=gather_queue(),
                                single_packet=False)
                            ccol += n_i // 16

                        # ---- d12 expansion (ACT), s-major run layout ----
                        d12e = epool.tile([128, J * 8], bf16, tag="d12e")
                        d12e3 = d12e[:].rearrange("p (j v) -> p j v", v=8)
                        for s in range(4):
                            for (t, K, ko) in tiles_here:
                                if K == 0:
                                    continue
                                o = s * KU + ko
                                nc.scalar.copy(
                                    d12e3[:, o:o + K, :],
                                    d12[:, t * 8:(t + 1) * 8].unsqueeze(
                                        1).to_broadcast([128, K, 8]))
                        # ---- t12 = E12_src * D12_dst (one flat op) ----
                        t12 = epool.tile([128, J * 8], bf16, tag="t12")
                        t123 = t12[:].rearrange("p (j v) -> p j v", v=8)
                        nc.vector.tensor_tensor(
                            out=t123[:, :, :], in0=g3[:, :, 68:76],
                            in1=d12e3[:, :, :], op=ALU.mult)
                        # ---- hcopy frees gat early (4x copy) ----
                        e64 = vpool.tile([128, J * 64], bf16, tag="e64")
                        vbuf = vpool.tile([128, J * 64], bf16, tag="vbuf")
                        nc.vector.tensor_copy(
                            vbuf[:].rearrange("p (j c) -> p j c", c=64),
                            g3[:, :, 0:64])
                        # ---- eb = max of halves ----
                        e_b = epool.tile([128, J * 4], bf16, tag="eb")
                        eb3 = e_b[:].rearrange("p (j q) -> p j q", q=4)
                        nc.vector.tensor_tensor(out=eb3[:, :, :],
                                                in0=t123[:, :, 0:4],
                                                in1=t123[:, :, 4:8],
                                                op=ALU.max)
                        # ---- e64 = ACT-expanded eb (16x bcast) ----
                        nc.scalar.copy(
                            e64[:].rearrange("p (a c) -> p a c", c=16),
                            e_b[:].unsqueeze(2).to_broadcast(
                                [128, J * 4, 16]))
                        # ---- v = h * e64 (in place on vbuf, flat 2x) ----
                        nc.vector.tensor_tensor(
                            out=vbuf[:], in0=vbuf[:], in1=e64[:],
                            op=ALU.mult)
                        # ---- fold the 4 s-blocks (values + denoms) ----
                        BV = KU * 64
                        nc.vector.tensor_tensor(
                            out=vbuf[:, 0:2 * BV], in0=vbuf[:, 0:2 * BV],
                            in1=vbuf[:, 2 * BV:4 * BV], op=ALU.add)
                        nc.vector.tensor_tensor(
                            out=vbuf[:, 0:BV], in0=vbuf[:, 0:BV],
                            in1=vbuf[:, BV:2 * BV], op=ALU.add)
                        B4 = KU * 4
                        den = epool.tile([128, 2 * B4], bf16, tag="den")
                        nc.vector.tensor_tensor(
                            out=den[:], in0=e_b[:, 0:2 * B4],
                            in1=e_b[:, 2 * B4:4 * B4], op=ALU.add)
                        nc.vector.tensor_tensor(
                            out=den[:, 0:B4], in0=den[:, 0:B4],
                            in1=den[:, B4:2 * B4], op=ALU.add)
                        dqj = den[:].rearrange("p (j q) -> p q j", q=4)

                    # ---- per tile: fold value columns, reduce ----
                    U = epool.tile([128, npr * 64], f32, tag="U")
                    dful = epool.tile([128, npr * 4], f32, tag="dful")
                    for i, (t, K, ko) in enumerate(tiles_here):
                        if J > 0 and K > 0:
                            b0 = ko
                            n = K
                            while n > 4:
                                half = n // 2
                                nc.vector.tensor_tensor(
                                    out=vbuf[:, b0 * 64:(b0 + half) * 64],
                                    in0=vbuf[:, b0 * 64:(b0 + half) * 64],
                                    in1=vbuf[:, (b0 + n - half) * 64:
                                            (b0 + n) * 64],
                                    op=ALU.add)
                                n -= half
                            v3c = vbuf[:].rearrange("p (j c) -> p c j", c=64)
                            nc.vector.tensor_reduce(
                                U[:, 64 * i:64 * i + 64],
                                v3c[:, :, b0:b0 + n], AX.X, ALU.add)
                            nc.vector.tensor_tensor(
                                out=U[:, 64 * i:64 * i + 64],
                                in0=U[:, 64 * i:64 * i + 64],
                                in1=sv_all[:, t * 64:(t + 1) * 64],
                                op=ALU.add)
                            nc.vector.tensor_reduce(
                                dful[:, 4 * i:4 * i + 4],
                                dqj[:, :, b0:b0 + K], AX.X, ALU.add)
                            nc.vector.tensor_tensor(
                                out=dful[:, 4 * i:4 * i + 4],
                                in0=dful[:, 4 * i:4 * i + 4],
                                in1=es_all[:, t * 4:(t + 1) * 4],
                                op=ALU.add)
                        else:
                            nc.vector.tensor_copy(
                                U[:, 64 * i:64 * i + 64],
                                sv_all[:, t * 64:(t + 1) * 64])
                            nc.vector.tensor_copy(
                                dful[:, 4 * i:4 * i + 4],
                                es_all[:, t * 4:(t + 1) * 4])

                    if pend is not None:
                        emit_post(*pend)
                    pend = (ui, tiles_here, npr, U, dful)
                emit_post(*pend)
                pend = None

            # ---------------- pooling + MLP head ----------------
            if run_layers == 3:
                pooled = hpool.tile([16, G], f32, tag="pooled")
                nc.scalar.copy(pooled[:], pool_ps[:])
                nc.sync.dma_start(cc_in[:, :], pooled[:])
                nc.gpsimd.collective_compute(
                    "AllReduce", mybir.AluOpType.add,
                    replica_groups=[list(range(NC))],
                    ins=[cc_in.opt()], outs=[cc_out.opt()])
                zt = hpool.tile([32, G], f32, tag="zt")
                nc.sync.dma_start(zt[0:16, :], cc_out[:, :])
                cr = hpool.tile([16, G], f32, tag="cr")
                nc.sync.dma_start(cr[:], cntr[:, :])
                nc.vector.tensor_tensor(out=zt[0:16, :], in0=zt[0:16, :],
                                        in1=cr[:], op=ALU.mult)
                nc.sync.dma_start(zt[16:32, :], statsT[:, :])
                fw1s = hpool.tile([32, 32], f32, tag="fw1")
                nc.sync.dma_start(fw1s[:], fw1[:, :])
                fb1s = hpool.tile([32, 1], f32, tag="fb1")
                nc.sync.dma_start(fb1s[:], fb1[:, :])
                fw2s = hpool.tile([32, 16], f32, tag="fw2")
                nc.sync.dma_start(fw2s[:], fw2[:, :])
                fb2s = hpool.tile([16, 1], f32, tag="fb2")
                nc.sync.dma_start(fb2s[:], fb2[:, :])
                fw3s = hpool.tile([16, 1], f32, tag="fw3")
                nc.sync.dma_start(fw3s[:], fw3[:, :])
                fb3s = hpool.tile([1, 1], f32, tag="fb3")
                nc.sync.dma_start(fb3s[:], fb3[:, :])

                mp1 = mpool.tile([32, G], f32, tag="mp1")
                nc.tensor.matmul(mp1[:], fw1s[:], zt[:], start=True,
                                 stop=True)
                h1 = hpool.tile([32, G], f32, tag="h1")
                nc.scalar.activation(h1[:], mp1[:], ACT.Relu,
                                     bias=fb1s[:, 0:1])
                mp2 = mpool.tile([16, G], f32, tag="mp2")
                nc.tensor.matmul(mp2[:], fw2s[:], h1[:], start=True,
                                 stop=True)
                h2 = hpool.tile([16, G], f32, tag="h2")
                nc.scalar.activation(h2[:], mp2[:], ACT.Relu,
                                     bias=fb2s[:, 0:1])
                mp3 = mpool.tile([1, G], f32, tag="mp3")
                nc.tensor.matmul(mp3[:], fw3s[:], h2[:], start=True,
                                 stop=True)
                ot = hpool.tile([1, G], f32, tag="ot")
                nc.vector.tensor_tensor(
                    out=ot[:], in0=mp3[:],
                    in1=fb3s[:, 0:1].to_broadcast([1, G]), op=ALU.add)
                nc.sync.dma_start(out_t[:, :], ot[:])

    nc.finalize()
    return nc


# ------------------------------------------------------------------- driver

def run_gat(x, stats, W1, a1s, a1d, b1, W2, a2s, a2d, b2, W3, a3s, a3d, b3,
            fw1, fb1, fw2, fb2, fw3, fb3, edge_index, batch,
            trace=False, _cache={}):
    from concourse.bass_utils import run_bass_kernel_spmd

    x = np.asarray(x, np.float32)
    stats = np.asarray(stats, np.float32)
    n_graphs = stats.shape[0]
    f_in = x.shape[1]
    meta = _prep(x, np.asarray(edge_index), np.asarray(batch), n_graphs)
    NC, PC, NSTAR = meta["NC"], meta["PC"], meta["NSTAR"]

    nc = _build(meta, n_graphs, f_in)

    pi = meta["pi_of"][:x.shape[0]]
    xs = np.zeros((NSTAR, f_in), np.float32)
    xs[pi] = x
    xT_full = np.ascontiguousarray(xs.reshape(NC, PC, f_in)
                                   .transpose(0, 2, 1)).astype(BF16)

    cntrep = np.tile((1.0 / meta["counts"]).astype(np.float32)[None, :],
                     (16, 1))
    in_common = dict(
        w1=_augment_w(np.asarray(W1, np.float32), np.asarray(a1s, np.float32),
                      np.asarray(a1d, np.float32)),
        w2=_augment_w(np.asarray(W2, np.float32), np.asarray(a2s, np.float32),
                      np.asarray(a2d, np.float32)),
        w3=_augment_w(np.asarray(W3, np.float32), np.asarray(a3s, np.float32),
                      np.asarray(a3d, np.float32)),
        b1r=np.tile(np.asarray(b1, np.float32)[None, :], (128, 2)),
        b2r=np.tile(np.asarray(b2, np.float32)[None, :], (128, 2)),
        b3r=np.tile(np.asarray(b3, np.float32)[None, :], (128, 2)),
        cntr=cntrep.astype(np.float32),
        statsT=np.ascontiguousarray(stats.T).astype(np.float32),
        fw1=np.asarray(fw1, np.float32),
        fb1=np.asarray(fb1, np.float32).reshape(32, 1),
        fw2=np.asarray(fw2, np.float32),
        fb2=np.asarray(fb2, np.float32).reshape(16, 1),
        fw3=np.asarray(fw3, np.float32),
        fb3=np.asarray(fb3, np.float32).reshape(1, 1),
        dumr=np.zeros((1, 128), np.float32).astype(BF16),
    )
    in_maps = []
    TOTC = meta["idx_all"].shape[2]
    for c in range(NC):
        m = dict(in_common)
        m["xT"] = np.ascontiguousarray(xT_full[c])
        ia = meta["idx_all"][c]
        if TOTC < 8:
            ia = np.zeros((128, 8), np.int16)
        m["idx"] = np.ascontiguousarray(ia)
        m["goh"] = meta["goh"][c].astype(BF16)
        in_maps.append(m)

    res = run_bass_kernel_spmd(nc, in_maps, list(range(NC)), trace=trace)
    out = res.results[0]["out"]
    return np.ascontiguousarray(out.T).astype(np.float32), res


def kernel(**inputs):
    out, _ = run_gat(**inputs)
    return out


# revision 9
# speedup vs baseline: 1.0554x; 1.0554x over previous
"""GAT (3-layer, PyG-style) forward on 8 Trainium2 NeuronCores.

v4 strategy (quarter-tables):
  - Nodes assigned to cores by id%8; each core's rank space [0,PC) split in 4
    quarters of Q ranks. Sources are greedily "colored" into quarters to
    balance each dst's in-edges across quarters; dsts are z-order clustered
    within (core, quarter) by their 4-vector quarter-degree profile.
  - Per layer, per quarter q, a Shared table tabq[l][q] = AllGather over cores
    of that quarter's staged rows ([h 64|AD 4|E1 4|E2 4] bf16, 256B rows).
    Chunk AGs fire progressively inside the previous layer's edge loop, so
    only the last chunk is exposed. Gather windows = one whole quarter-table
    (8*QB rows <= 32767, int16-addressable).
  - Tiles are paired K-balanced within each quarter; per tile the slot count
    K is uniform across the 4 quarters, so a unit's gathered slots form 4
    equal s-blocks -> all hot DVE ops are flat unit-stride bf16 (2x mode):
    one t12 mult (vs ACT-expanded D12), one max, block-folds for value and
    denominator sums. The per-slot e is ACT-expanded to 64 wide, and the
    value multiply runs in-place at 2x.
  - exp(leaky_relu(as+ad)) == max(exp(as)exp(ad), exp(.2as)exp(.2ad)).
  - Self contributions precomputed densely per layer; per-unit post ops,
    next-layer node matmul fused; final pooling matmul + AllReduce + MLP.
"""

import sys

sys.path.insert(0, "/opt/trn_rl_repo")

import numpy as np
import ml_dtypes

BF16 = ml_dtypes.bfloat16

MAXJ_CALL = 28  # <=3584 idx per dma_gather (ring w/ 64KB scratch)
NQ = 4          # SWDGE queues
SW = 76         # staged row cols [h 64|AD 4|E1 4|E2 4]


# ----------------------------------------------------------------- host prep

def _color_sources(src, dst, NSTAR, rng):
    """Greedy 4-coloring of sources balancing per-dst quarter counts."""
    E = len(src)
    order_e = np.argsort(src, kind='stable')
    src_s, dst_s = src[order_e], dst[order_e]
    uniq, starts = np.unique(src_s, return_index=True)
    ends = np.append(starts[1:], E)
    outdeg_u = ends - starts
    proc = np.argsort(-outdeg_u, kind='stable')
    d_tot = np.bincount(dst, minlength=NSTAR)
    ideal = (d_tot + 3) // 4

    cnt = np.zeros((NSTAR, 4), np.int32)
    color = (np.arange(NSTAR) % 4).astype(np.int8)
    for i in proc:
        u = uniq[i]
        ds = dst_s[starts[i]:ends[i]]
        c_over = np.maximum(cnt[ds] + 1 - ideal[ds][:, None], 0).sum(0)
        c = int(np.argmin(c_over * 1000 + cnt[ds].sum(0)))
        color[u] = c
        cnt[ds, c] += 1
    for _ in range(2):
        for i in proc:
            u = uniq[i]
            ds = dst_s[starts[i]:ends[i]]
            cnt[ds, color[u]] -= 1
            c_over = np.maximum(cnt[ds] + 1 - ideal[ds][:, None], 0).sum(0)
            c = int(np.argmin(c_over * 1000 + cnt[ds].sum(0)))
            color[u] = c
            cnt[ds, c] += 1
    return color


def _zorder(prof):
    p = np.minimum(prof, 31).astype(np.int64)
    z = np.zeros(len(p), np.int64)
    for bit in range(5):
        for d in range(4):
            z |= ((p[:, d] >> bit) & 1) << (bit * 4 + d)
    return np.argsort(z, kind='stable')


def _prep(x, edge_index, batch, n_graphs):
    N = x.shape[0]
    NC = 8
    PC = int(np.ceil(N / NC / 512)) * 512
    NSTAR = NC * PC
    TILES = PC // 128
    Q = PC // 4
    QB = Q + 128
    TPQ = TILES // 4
    DUMMY_LOC = Q                     # row Q of core-0 block in each tabq

    src = edge_index[0].astype(np.int64)
    dst = edge_index[1].astype(np.int64)
    rng = np.random.default_rng(0)

    core_of = np.full(NSTAR, -1, np.int64)
    core_of[:N] = np.arange(N) % NC

    color = _color_sources(src, dst, NSTAR, rng)

    # per (core, color) capacity Q: rebalance moving low-outdeg nodes
    odeg_all = np.bincount(src, minlength=NSTAR)
    for c in range(NC):
        ids = np.where(core_of == c)[0]
        cols = color[ids].astype(np.int64)
        for _ in range(12):
            cc = np.bincount(cols, minlength=4)
            if (cc <= Q).all():
                break
            over = int(np.argmax(cc))
            under = int(np.argmin(cc))
            n_move = min(cc[over] - Q, Q - cc[under])
            cand = np.where(cols == over)[0]
            take = cand[np.argsort(odeg_all[ids[cand]])[:n_move]]
            cols[take] = under
        color[ids] = cols

    degq = np.zeros((NSTAR, 4), np.int64)
    np.add.at(degq, (dst, color[src]), 1)

    # rank within (core, quarter): z-order of profile; pads fill the rest
    rank_of = np.full(NSTAR, -1, np.int64)
    for c in range(NC):
        ids = np.where(core_of == c)[0]
        cols = color[ids]
        for q in range(4):
            sub = ids[cols == q]
            o = _zorder(degq[sub])
            rank_of[sub[o]] = q * Q + np.arange(len(sub))

    # per-core-tile per-s max, then cross-core & cross-s max -> K_round[t]
    tile_K = np.zeros((NC, TILES, 4), np.int64)
    for c in range(NC):
        ids = np.where(core_of == c)[0]
        t = rank_of[ids] // 128
        prof = degq[ids]
        for s in range(4):
            np.maximum.at(tile_K[c, :, s], t, prof[:, s])
    K_round = tile_K.max(axis=(0, 2))          # [TILES], s-uniform

    # units: K-balanced pairing within each quarter
    units = []                                 # (tA, tB|None, KA, KB)
    unit_quarter = []
    for q in range(4):
        tq = np.arange(q * TPQ, (q + 1) * TPQ)
        order = tq[np.argsort(K_round[tq], kind='stable')]
        for i in range(TPQ // 2):
            tA, tB = int(order[i]), int(order[TPQ - 1 - i])
            units.append((tA, tB, int(K_round[tA]), int(K_round[tB])))
            unit_quarter.append(q)
        if TPQ % 2:
            tA = int(order[TPQ // 2])
            units.append((tA, None, int(K_round[tA]), 0))
            unit_quarter.append(q)
    NU = len(units)
    # AG trigger unit index per quarter (last unit of each quarter)
    ag_after = {}
    for ui, q in enumerate(unit_quarter):
        ag_after[q] = ui

    # ---- slot lists ------------------------------------------------------
    slots = [[np.full((128, 4 * int(K_round[t])), DUMMY_LOC, np.int16)
              for t in range(TILES)] for c in range(NC)]
    # layout per tile: s-major [s0 K | s1 K | s2 K | s3 K]
    dcore = core_of[dst]
    drank = rank_of[dst]
    scol = color[src].astype(np.int64)
    loc = (core_of[src] * QB + (rank_of[src] - scol * Q)).astype(np.int64)
    assert loc.min() >= 0 and loc.max() < NC * QB <= 32767
    order = np.lexsort((scol, drank, dcore))
    kfill = np.zeros((NSTAR, 4), np.int64)
    dco, dro, sco, lco, dsto = (dcore[order], drank[order], scol[order],
                                loc[order], dst[order])
    Kt = K_round.astype(np.int64)
    for i in range(len(dco)):
        c = dco[i]
        r = dro[i]
        t = r // 128
        p = r % 128
        s = sco[i]
        k = kfill[dsto[i], s]
        slots[c][t][p, s * Kt[t] + k] = lco[i]
        kfill[dsto[i], s] = k + 1

    # ---- calls (uniform across cores) -----------------------------------
    calls = []           # (ui, s, joff_unit, cj)
    unit_J = []
    for ui, (tA, tB, KA, KB) in enumerate(units):
        KU = KA + KB
        for s in range(4):
            o = 0
            while o < KU:
                cj = min(MAXJ_CALL, KU - o)
                calls.append((ui, s, s * KU + o, cj))
                o += cj
        unit_J.append(4 * KU)

    def wrap16(ix):
        a = ix.reshape(-1, 16).T
        return np.tile(a, (8, 1))

    tile_cols = []       # (idx col offset, ncols) per unit
    off = 0
    for ui in range(NU):
        ncols = 128 * unit_J[ui] // 16
        tile_cols.append((off, ncols))
        off += ncols

    idx_cores = []
    for c in range(NC):
        parts = []
        for (ui, s, joff, cj) in calls:
            tA, tB, KA, KB = units[ui]
            KU = KA + KB
            blkA = slots[c][tA][:, s * KA:(s + 1) * KA]
            if tB is not None:
                blkB = slots[c][tB][:, s * KB:(s + 1) * KB]
                blk = np.concatenate([blkA, blkB], axis=1)
            else:
                blk = blkA
            js = joff - s * KU
            sub = blk[:, js:js + cj]
            ix = sub.T.reshape(-1).astype(np.int16)
            parts.append(wrap16(ix).astype(np.int16))
        idx_cores.append(np.concatenate(parts, axis=1)
                         if parts else np.zeros((128, 0), np.int16))
    idx_all = np.stack(idx_cores)

    pi_of = np.full(NSTAR, -1, np.int64)
    pi_of[:N] = core_of[:N] * PC + rank_of[:N]
    inv_pi = np.full(NSTAR, -1, np.int64)
    inv_pi[pi_of[:N]] = np.arange(N)

    batch = np.asarray(batch, np.int64)
    goh = np.zeros((NC, PC, n_graphs), np.float32)
    for c in range(NC):
        b = np.full(PC, -1, np.int64)
        iv = inv_pi[c * PC:(c + 1) * PC]
        m = iv >= 0
        b[m] = batch[iv[m]]
        valid = b >= 0
        goh[c, np.arange(PC)[valid], b[valid]] = 1.0
    counts = np.maximum(np.bincount(batch, minlength=n_graphs), 1.0)

    return dict(
        N=N, NC=NC, PC=PC, NSTAR=NSTAR, TILES=TILES, Q=Q, QB=QB, TPQ=TPQ,
        K_round=K_round, units=units, unit_J=unit_J, calls=calls,
        tile_cols=tile_cols, idx_all=idx_all, inv_pi=inv_pi, pi_of=pi_of,
        goh=goh, counts=counts, ag_after=ag_after,
    )


def _augment_w(W, a_s, a_d, heads=4, hid=16):
    """[F, H*C] weights -> [F, 72] augmented (bf16): [W | Wad | Was]."""
    F = W.shape[0]
    Wr = W.reshape(F, heads, hid)
    was = np.einsum("fhc,hc->fh", Wr, a_s)
    wad = np.einsum("fhc,hc->fh", Wr, a_d)
    out = np.concatenate([W, wad, was], axis=1).astype(np.float32)
    return out.astype(BF16)


# ------------------------------------------------------------- kernel build

def _build(meta, n_graphs, f_in, run_layers=3):
    import concourse.tile as tile
    from concourse import bacc, mybir
    from concourse.masks import make_identity

    NC, PC, TILES = meta["NC"], meta["PC"], meta["TILES"]
    Q, QB, TPQ = meta["Q"], meta["QB"], meta["TPQ"]
    TROWS = NC * QB
    units, unit_J, calls = meta["units"], meta["unit_J"], meta["calls"]
    tile_cols, ag_after = meta["tile_cols"], meta["ag_after"]
    NU = len(units)
    TOTC = meta["idx_all"].shape[2]
    G = n_graphs
    f32, bf16, i16 = mybir.dt.float32, mybir.dt.bfloat16, mybir.dt.int16
    AX, ALU = mybir.AxisListType, mybir.AluOpType
    ACT = mybir.ActivationFunctionType

    nc = bacc.Bacc(None, target_bir_lowering=False, debug=False,
                   num_devices=NC, num_swdge_queues=NQ,
                   dynamic_dma_scratch_size=49152)

    # ---- I/O ----
    xT = nc.dram_tensor("xT", [f_in, PC], bf16, kind="ExternalInput")
    idx_in = nc.dram_tensor("idx", [128, max(TOTC, 8)], i16,
                            kind="ExternalInput")
    goh_in = nc.dram_tensor("goh", [PC, G], bf16, kind="ExternalInput")
    w1 = nc.dram_tensor("w1", [f_in, 72], bf16, kind="ExternalInput")
    w2 = nc.dram_tensor("w2", [64, 72], bf16, kind="ExternalInput")
    w3 = nc.dram_tensor("w3", [16, 72], bf16, kind="ExternalInput")
    b1r = nc.dram_tensor("b1r", [128, 128], f32, kind="ExternalInput")
    b2r = nc.dram_tensor("b2r", [128, 32], f32, kind="ExternalInput")
    b3r = nc.dram_tensor("b3r", [128, 32], f32, kind="ExternalInput")
    cntr = nc.dram_tensor("cntr", [16, G], f32, kind="ExternalInput")
    statsT = nc.dram_tensor("statsT", [16, G], f32, kind="ExternalInput")
    fw1 = nc.dram_tensor("fw1", [32, 32], f32, kind="ExternalInput")
    fb1 = nc.dram_tensor("fb1", [32, 1], f32, kind="ExternalInput")
    fw2 = nc.dram_tensor("fw2", [32, 16], f32, kind="ExternalInput")
    fb2 = nc.dram_tensor("fb2", [16, 1], f32, kind="ExternalInput")
    fw3 = nc.dram_tensor("fw3", [16, 1], f32, kind="ExternalInput")
    fb3 = nc.dram_tensor("fb3", [1, 1], f32, kind="ExternalInput")
    dumr = nc.dram_tensor("dumr", [1, 128], bf16, kind="ExternalInput")
    out_t = nc.dram_tensor("out", [1, G], f32, kind="ExternalOutput")

    LIN = [f_in, 64, 16]
    LOUT = [64, 16, 16]

    with tile.TileContext(nc, num_cores=NC) as tc:
        with (
            tc.tile_pool(name="dram", bufs=1, space="DRAM") as dpool,
            tc.tile_pool(name="consts", bufs=1) as cpool,
            tc.tile_pool(name="nodein", bufs=2) as npool,
            tc.tile_pool(name="psum", bufs=2, space="PSUM") as ppool,
            tc.tile_pool(name="pst", bufs=2, space="PSUM") as ptpool,
            tc.tile_pool(name="mlpp", bufs=1, space="PSUM") as mpool,
            tc.tile_pool(name="stall", bufs=1) as stpool,
            tc.tile_pool(name="selfp", bufs=1) as sepool,
            tc.tile_pool(name="gat", bufs=3) as gpool,
            tc.tile_pool(name="e64p", bufs=2) as vpool,
            tc.tile_pool(name="idxp", bufs=4) as ipool,
            tc.tile_pool(name="edge", bufs=3) as epool,
            tc.tile_pool(name="escr", bufs=2) as e1pool,
            tc.tile_pool(name="poolacc", bufs=1, space="PSUM") as papool,
            tc.tile_pool(name="head", bufs=1) as hpool,
        ):
            tabmine = [dpool.tile([4 * QB, 128], bf16, tag=f"tm{l}",
                                  name=f"tabmine{l}") for l in range(3)]
            tabq = [[dpool.tile([TROWS, 128], bf16, tag=f"tab{l}_{s}",
                                name=f"table{l}_{s}", addr_space="Shared")
                     for s in range(4)] for l in range(3)]
            cc_in = dpool.tile([16, G], f32, tag="ccin")
            cc_out = dpool.tile([16, G], f32, tag="ccout",
                                addr_space="Shared")

            ident = cpool.tile([128, 128], bf16)
            make_identity(nc, ident[:])
            wsb = []
            for l, wt in enumerate((w1, w2, w3)):
                t = cpool.tile([LIN[l], 72], bf16, tag=f"w{l}", name=f"wsb{l}")
                nc.sync.dma_start(t[:], wt[:, :])
                wsb.append(t)
            brep = []
            for l, (bt, w2_) in enumerate(((b1r, 128), (b2r, 32), (b3r, 32))):
                t = cpool.tile([128, w2_], f32, tag=f"b{l}", name=f"bsb{l}")
                nc.sync.dma_start(t[:], bt[:, :])
                brep.append(t)
            dum_sb = cpool.tile([1, 128], bf16)
            nc.sync.dma_start(dum_sb[:], dumr[:, :])

            st_bufs = [stpool.tile([128, TILES * SW], bf16, tag=f"sta{l}",
                                   name=f"stall{l}") for l in range(2)]
            st_all = [st_bufs[0], st_bufs[1], st_bufs[0]]

            pool_ps = papool.tile([16, G], f32)
            goh_sb = stpool.tile([128, TILES * G], bf16, tag="gohsb")
            gview = goh_in[:].rearrange("(t p) g -> p t g", p=128)
            nc.sync.dma_start(
                goh_sb[:].rearrange("p (t g) -> p t g", g=G), gview)

            qctr = [0]

            def gather_queue():
                q = qctr[0] % NQ
                qctr[0] += 1
                return q

            def elu_out(x_ap, out_ap, w):
                # elu(x) = relu(x) + exp(-relu(-x)) - 1
                t1 = e1pool.tile([128, w], bf16, tag="el1")
                t2 = e1pool.tile([128, w], bf16, tag="el2")
                nc.scalar.activation(t1[:], x_ap, ACT.Relu, scale=-1.0)
                nc.scalar.activation(t1[:], t1[:], ACT.Exp, scale=-1.0)
                nc.scalar.activation(t2[:], x_ap, ACT.Relu)
                nc.vector.tensor_scalar_add(t1[:], t1[:], -1.0)
                nc.vector.tensor_tensor(out=out_ap, in0=t1[:], in1=t2[:],
                                        op=ALU.add)

            def node_tile(l, t, lhsT_ap):
                """row_l[tile t] = [h|AD|E1|E2] -> st_all[l] + tabmine[l]."""
                ps = ppool.tile([128, 72], f32, tag="nps")
                nc.tensor.matmul(ps[:], lhsT_ap, wsb[l][:],
                                 start=True, stop=True)
                sl = st_all[l][:, t * SW:(t + 1) * SW]
                nc.scalar.copy(sl[:, 0:68], ps[:, 0:68])
                nc.scalar.activation(sl[:, 68:72], ps[:, 68:72], ACT.Exp)
                nc.scalar.activation(sl[:, 72:76], ps[:, 68:72], ACT.Exp,
                                     scale=0.2)
                q = t // TPQ
                row0 = q * QB + (t % TPQ) * 128
                nc.sync.dma_start(tabmine[l][row0:row0 + 128, 0:SW], sl)

            def ag_chunk(l, q):
                nc.sync.dma_start(tabmine[l][q * QB + Q:q * QB + Q + 1, :],
                                  dum_sb[:])
                nc.gpsimd.collective_compute(
                    "AllGather", mybir.AluOpType.bypass,
                    replica_groups=[list(range(NC))],
                    ins=[tabmine[l][q * QB:(q + 1) * QB, :]],
                    outs=[tabq[l][q].opt()])

            # ---------------- layer 0 node phase --------------------------
            for q in range(4):
                t0, t1_ = q * TPQ, (q + 1) * TPQ
                xin = npool.tile([f_in, TPQ * 128], bf16, tag="xin")
                nc.sync.dma_start(xin[:], xT[:, t0 * 128:t1_ * 128])
                for t in range(t0, t1_):
                    node_tile(0, t, xin[:, (t - t0) * 128:(t - t0 + 1) * 128])
                ag_chunk(0, q)

            for l in range(run_layers):
                W = LOUT[l]
                sa = st_all[l]
                sa3 = sa[:].rearrange("p (r c) -> p r c", c=SW)

                # ---- bulk self/dst precompute --------------------------
                d12 = sepool.tile([128, TILES * 8], bf16, tag="d12")
                d123 = d12[:].rearrange("p (r v) -> p r v", v=8)
                nc.scalar.activation(d123[:, :, 0:4], sa3[:, :, 64:68],
                                     ACT.Exp)
                nc.scalar.activation(d123[:, :, 4:8], sa3[:, :, 64:68],
                                     ACT.Exp, scale=0.2)
                est = sepool.tile([128, TILES * 8], bf16, tag="est")
                est3 = est[:].rearrange("p (r v) -> p r v", v=8)
                nc.vector.tensor_tensor(out=est3[:, :, :],
                                        in0=sa3[:, :, 68:76],
                                        in1=d123[:, :, :], op=ALU.mult)
                es_all = sepool.tile([128, TILES * 4], f32, tag="esal")
                es3 = es_all[:].rearrange("p (r q) -> p r q", q=4)
                nc.vector.tensor_tensor(out=es3[:, :, :],
                                        in0=est3[:, :, 0:4],
                                        in1=est3[:, :, 4:8], op=ALU.max)
                sv_all = sepool.tile([128, TILES * 64], bf16, tag="sval")
                sv4 = sv_all[:].rearrange("p (r q c) -> p r q c", q=4, c=16)
                nc.vector.tensor_tensor(
                    out=sv4[:, :, :, :],
                    in0=sa3[:, :, 0:64].rearrange("p r (q c) -> p r q c",
                                                  c=16),
                    in1=es3.unsqueeze(3).to_broadcast([128, TILES, 4, 16]),
                    op=ALU.mult)

                # ---------------- edge phase ----------------------------
                def emit_post(ui, tiles_here, npr, U, dful):
                    recip = epool.tile([128, npr * 4], f32, tag="rec")
                    nc.vector.reciprocal(recip[:], dful[:])
                    if l > 0:
                        nc.vector.tensor_scalar_mul(recip[:], recip[:], 0.25)
                    o64 = epool.tile([128, npr * 64], f32, tag="o64")
                    nc.vector.tensor_tensor(
                        out=o64[:].rearrange("p (t q c) -> p t q c",
                                             q=4, c=16),
                        in0=U[:].rearrange("p (t q c) -> p t q c",
                                           q=4, c=16),
                        in1=recip[:].rearrange(
                            "p (t q) -> p t q", q=4).unsqueeze(
                            3).to_broadcast([128, npr, 4, 16]),
                        op=ALU.mult)
                    if l == 0:
                        nc.vector.tensor_tensor(
                            out=o64[:], in0=o64[:],
                            in1=brep[0][:, 0:npr * 64], op=ALU.add)
                        xnext = epool.tile([128, npr * 64], bf16, tag="xn")
                        elu_out(o64[:], xnext[:], npr * 64)
                        WX = 64
                    else:
                        o16 = epool.tile([128, npr * 16], f32, tag="o16")
                        nc.vector.tensor_reduce(
                            o16[:],
                            o64[:].rearrange("p (t q c) -> p t c q",
                                             q=4, c=16),
                            AX.X, ALU.add)
                        xnext = epool.tile([128, npr * 16], bf16, tag="xn16")
                        if l == 1:
                            nc.vector.tensor_tensor(
                                out=o16[:], in0=o16[:],
                                in1=brep[l][:, 0:npr * 16], op=ALU.add)
                            elu_out(o16[:], xnext[:], npr * 16)
                        else:
                            nc.vector.tensor_tensor(
                                out=xnext[:], in0=o16[:],
                                in1=brep[l][:, 0:npr * 16], op=ALU.add)
                        WX = 16

                    if l < 2:
                        for i, (t, K, ko) in enumerate(tiles_here):
                            pst = ptpool.tile([WX, 128], bf16, tag="pst")
                            nc.tensor.transpose(
                                out=pst[:],
                                in_=xnext[:, WX * i:WX * (i + 1)],
                                identity=ident[:])
                            stt = epool.tile([WX, 128], bf16, tag="stt")
                            nc.scalar.copy(stt[:], pst[:])
                            if run_layers > l + 1:
                                node_tile(l + 1, t, stt[:])
                    else:
                        for i, (t, K, ko) in enumerate(tiles_here):
                            nc.tensor.matmul(
                                pool_ps[:], xnext[:, 16 * i:16 * i + 16],
                                goh_sb[:, t * G:(t + 1) * G],
                                start=(ui == 0 and i == 0),
                                stop=(ui == NU - 1 and i == npr - 1))

                    if l < 2 and run_layers > l + 1:
                        for q in range(4):
                            if ag_after[q] == ui:
                                ag_chunk(l + 1, q)

                pend = None
                for ui in range(NU):
                    tA, tB, KA, KB = units[ui]
                    KU = KA + KB
                    J = 4 * KU
                    tiles_here = [(tA, KA, 0)] if tB is None else \
                        [(tA, KA, 0), (tB, KB, KA)]
                    npr = len(tiles_here)
                    coff, ncols = tile_cols[ui]

                    if J > 0:
                        it = ipool.tile([128, max(ncols, 8)], i16, tag="idx")
                        nc.sync.dma_start(it[:, 0:ncols],
                                          idx_in[:, coff:coff + ncols])
                        gat = gpool.tile([128, J * 128], bf16, tag="gat")
                        g3 = gat[:].rearrange("p (j e) -> p j e", e=128)
                        ccol = 0
                        for (ui2, s_, joff, cj) in calls:
                            if ui2 != ui:
                                continue
                            n_i = 128 * cj
                            nc.gpsimd.dma_gather(
                                g3[:, joff:joff + cj, :],
                                tabq[l][s_][0:TROWS, :],
                                it[:, ccol:ccol + n_i // 16],
                                n_i, n_i, 128,
                                queue_num=gather_queue(),
                                single_packet=False)
                            ccol += n_i // 16

                        # ---- d12 expansion (ACT), s-major run layout ----
                        d12e = epool.tile([128, J * 8], bf16, tag="d12e")
                        d12e3 = d12e[:].rearrange("p (j v) -> p j v", v=8)
                        for s in range(4):
                            for (t, K, ko) in tiles_here:
                                if K == 0:
                                    continue
                                o = s * KU + ko
                                nc.scalar.copy(
                                    d12e3[:, o:o + K, :],
                                    d12[:, t * 8:(t + 1) * 8].unsqueeze(
                                        1).to_broadcast([128, K, 8]))
                        # ---- t12 = E12_src * D12_dst (one flat op) ----
                        t12 = epool.tile([128, J * 8], bf16, tag="t12")
                        t123 = t12[:].rearrange("p (j v) -> p j v", v=8)
                        nc.vector.tensor_tensor(
                            out=t123[:, :, :], in0=g3[:, :, 68:76],
                            in1=d12e3[:, :, :], op=ALU.mult)
                        # ---- hcopy frees gat early (4x copy) ----
                        e64 = vpool.tile([128, J * 64], bf16, tag="e64")
                        vbuf = vpool.tile([128, J * 64], bf16, tag="vbuf")
                        nc.vector.tensor_copy(
                            vbuf[:].rearrange("p (j c) -> p j c", c=64),
                            g3[:, :, 0:64])
                        # ---- eb = max of halves ----
                        e_b = epool.tile([128, J * 4], bf16, tag="eb")
                        eb3 = e_b[:].rearrange("p (j q) -> p j q", q=4)
                        nc.vector.tensor_tensor(out=eb3[:, :, :],
                                                in0=t123[:, :, 0:4],
                                                in1=t123[:, :, 4:8],
                                                op=ALU.max)
                        # ---- e64 = ACT-expanded eb (16x bcast) ----
                        nc.scalar.copy(
                            e64[:].rearrange("p (a c) -> p a c", c=16),
                            e_b[:].unsqueeze(2).to_broadcast(
                                [128, J * 4, 16]))
                        # ---- v = h * e64 (in place on vbuf, flat 2x) ----
                        nc.vector.tensor_tensor(
                            out=vbuf[:], in0=vbuf[:], in1=e64[:],
                            op=ALU.mult)
                        # ---- fold the 4 s-blocks (values + denoms) ----
                        BV = KU * 64
                        nc.vector.tensor_tensor(
                            out=vbuf[:, 0:2 * BV], in0=vbuf[:, 0:2 * BV],
                            in1=vbuf[:, 2 * BV:4 * BV], op=ALU.add)
                        nc.vector.tensor_tensor(
                            out=vbuf[:, 0:BV], in0=vbuf[:, 0:BV],
                            in1=vbuf[:, BV:2 * BV], op=ALU.add)
                        B4 = KU * 4
                        den = epool.tile([128, 2 * B4], bf16, tag="den")
                        nc.vector.tensor_tensor(
                            out=den[:], in0=e_b[:, 0:2 * B4],
                            in1=e_b[:, 2 * B4:4 * B4], op=ALU.add)
                        nc.vector.tensor_tensor(
                            out=den[:, 0:B4], in0=den[:, 0:B4],
                            in1=den[:, B4:2 * B4], op=ALU.add)
                        dqj = den[:].rearrange("p (j q) -> p q j", q=4)

                    # ---- per tile: fold value columns, reduce ----
                    U = epool.tile([128, npr * 64], f32, tag="U")
                    dful = epool.tile([128, npr * 4], f32, tag="dful")
                    for i, (t, K, ko) in enumerate(tiles_here):
                        if J > 0 and K > 0:
                            b0 = ko
                            n = K
                            while n > 4:
                                half = n // 2
                                nc.vector.tensor_tensor(
                                    out=vbuf[:, b0 * 64:(b0 + half) * 64],
                                    in0=vbuf[:, b0 * 64:(b0 + half) * 64],
                                    in1=vbuf[:, (b0 + n - half) * 64:
                                            (b0 + n) * 64],
                                    op=ALU.add)
                                n -= half
                            v3c = vbuf[:].rearrange("p (j c) -> p c j", c=64)
                            nc.vector.tensor_reduce(
                                U[:, 64 * i:64 * i + 64],
                                v3c[:, :, b0:b0 + n], AX.X, ALU.add)
                            nc.vector.tensor_tensor(
                                out=U[:, 64 * i:64 * i + 64],
                                in0=U[:, 64 * i:64 * i + 64],
                                in1=sv_all[:, t * 64:(t + 1) * 64],
                                op=ALU.add)
                            nc.vector.tensor_reduce(
                                dful[:, 4 * i:4 * i + 4],
                                dqj[:, :, b0:b0 + K], AX.X, ALU.add)
                            nc.vector.tensor_tensor(
                                out=dful[:, 4 * i:4 * i + 4],
                                in0=dful[:, 4 * i:4 * i + 4],
                                in1=es_all[:, t * 4:(t + 1) * 4],
                                op=ALU.add)
                        else:
                            nc.vector.tensor_copy(
                                U[:, 64 * i:64 * i + 64],
                                sv_all[:, t * 64:(t + 1) * 64])
                            nc.vector.tensor_copy(
                                dful[:, 4 * i:4 * i + 4],
                                es_all[:, t * 4:(t + 1) * 4])

                    if pend is not None:
                        emit_post(*pend)
                    pend = (ui, tiles_here, npr, U, dful)
                emit_post(*pend)
                pend = None

            # ---------------- pooling + MLP head ----------------
            if run_layers == 3:
                pooled = hpool.tile([16, G], f32, tag="pooled")
                nc.scalar.copy(pooled[:], pool_ps[:])
                nc.sync.dma_start(cc_in[:, :], pooled[:])
                nc.gpsimd.collective_compute(
                    "AllReduce", mybir.AluOpType.add,
                    replica_groups=[list(range(NC))],
                    ins=[cc_in.opt()], outs=[cc_out.opt()])
                zt = hpool.tile([32, G], f32, tag="zt")
                nc.sync.dma_start(zt[0:16, :], cc_out[:, :])
                cr = hpool.tile([16, G], f32, tag="cr")
                nc.sync.dma_start(cr[:], cntr[:, :])
                nc.vector.tensor_tensor(out=zt[0:16, :], in0=zt[0:16, :],
                                        in1=cr[:], op=ALU.mult)
                nc.sync.dma_start(zt[16:32, :], statsT[:, :])
                fw1s = hpool.tile([32, 32], f32, tag="fw1")
                nc.sync.dma_start(fw1s[:], fw1[:, :])
                fb1s = hpool.tile([32, 1], f32, tag="fb1")
                nc.sync.dma_start(fb1s[:], fb1[:, :])
                fw2s = hpool.tile([32, 16], f32, tag="fw2")
                nc.sync.dma_start(fw2s[:], fw2[:, :])
                fb2s = hpool.tile([16, 1], f32, tag="fb2")
                nc.sync.dma_start(fb2s[:], fb2[:, :])
                fw3s = hpool.tile([16, 1], f32, tag="fw3")
                nc.sync.dma_start(fw3s[:], fw3[:, :])
                fb3s = hpool.tile([1, 1], f32, tag="fb3")
                nc.sync.dma_start(fb3s[:], fb3[:, :])

                mp1 = mpool.tile([32, G], f32, tag="mp1")
                nc.tensor.matmul(mp1[:], fw1s[:], zt[:], start=True,
                                 stop=True)
                h1 = hpool.tile([32, G], f32, tag="h1")
                nc.scalar.activation(h1[:], mp1[:], ACT.Relu,
                                     bias=fb1s[:, 0:1])
                mp2 = mpool.tile([16, G], f32, tag="mp2")
                nc.tensor.matmul(mp2[:], fw2s[:], h1[:], start=True,
                                 stop=True)
                h2 = hpool.tile([16, G], f32, tag="h2")
                nc.scalar.activation(h2[:], mp2[:], ACT.Relu,
                                     bias=fb2s[:, 0:1])
                mp3 = mpool.tile([1, G], f32, tag="mp3")
                nc.tensor.matmul(mp3[:], fw3s[:], h2[:], start=True,
                                 stop=True)
                ot = hpool.tile([1, G], f32, tag="ot")
                nc.vector.tensor_tensor(
                    out=ot[:], in0=mp3[:],
                    in1=fb3s[:, 0:1].to_broadcast([1, G]), op=ALU.add)
                nc.sync.dma_start(out_t[:, :], ot[:])

    nc.finalize()
    return nc


# ------------------------------------------------------------------- driver

def run_gat(x, stats, W1, a1s, a1d, b1, W2, a2s, a2d, b2, W3, a3s, a3d, b3,
            fw1, fb1, fw2, fb2, fw3, fb3, edge_index, batch,
            trace=False, _cache={}):
    from concourse.bass_utils import run_bass_kernel_spmd

    x = np.asarray(x, np.float32)
    stats = np.asarray(stats, np.float32)
    n_graphs = stats.shape[0]
    f_in = x.shape[1]
    meta = _prep(x, np.asarray(edge_index), np.asarray(batch), n_graphs)
    NC, PC, NSTAR = meta["NC"], meta["PC"], meta["NSTAR"]

    nc = _build(meta, n_graphs, f_in)

    pi = meta["pi_of"][:x.shape[0]]
    xs = np.zeros((NSTAR, f_in), np.float32)
    xs[pi] = x
    xT_full = np.ascontiguousarray(xs.reshape(NC, PC, f_in)
                                   .transpose(0, 2, 1)).astype(BF16)

    cntrep = np.tile((1.0 / meta["counts"]).astype(np.float32)[None, :],
                     (16, 1))
    in_common = dict(
        w1=_augment_w(np.asarray(W1, np.float32), np.asarray(a1s, np.float32),
                      np.asarray(a1d, np.float32)),
        w2=_augment_w(np.asarray(W2, np.float32), np.asarray(a2s, np.float32),
                      np.asarray(a2d, np.float32)),
        w3=_augment_w(np.asarray(W3, np.float32), np.asarray(a3s, np.float32),
                      np.asarray(a3d, np.float32)),
        b1r=np.tile(np.asarray(b1, np.float32)[None, :], (128, 2)),
        b2r=np.tile(np.asarray(b2, np.float32)[None, :], (128, 2)),
        b3r=np.tile(np.asarray(b3, np.float32)[None, :], (128, 2)),
        cntr=cntrep.astype(np.float32),
        statsT=np.ascontiguousarray(stats.T).astype(np.float32),
        fw1=np.asarray(fw1, np.float32),
        fb1=np.asarray(fb1, np.float32).reshape(32, 1),
        fw2=np.asarray(fw2, np.float32),
        fb2=np.asarray(fb2, np.float32).reshape(16, 1),
        fw3=np.asarray(fw3, np.float32),
        fb3=np.asarray(fb3, np.float32).reshape(1, 1),
        dumr=np.zeros((1, 128), np.float32).astype(BF16),
    )
    in_maps = []
    TOTC = meta["idx_all"].shape[2]
    for c in range(NC):
        m = dict(in_common)
        m["xT"] = np.ascontiguousarray(xT_full[c])
        ia = meta["idx_all"][c]
        if TOTC < 8:
            ia = np.zeros((128, 8), np.int16)
        m["idx"] = np.ascontiguousarray(ia)
        m["goh"] = meta["goh"][c].astype(BF16)
        in_maps.append(m)

    res = run_bass_kernel_spmd(nc, in_maps, list(range(NC)), trace=trace)
    out = res.results[0]["out"]
    return np.ascontiguousarray(out.T).astype(np.float32), res


def kernel(**inputs):
    out, _ = run_gat(**inputs)
    return out
